# revision 120
# baseline (speedup 1.0000x reference)
"""Trainium2 Bass kernel for nn_BaseContextAwareModel (4-layer GCN + mean-pool + conv1d head).

Strategy (per the graph-id sharding hint):
- Each of the 1920 22-node frame-graphs is independent -> shard 240 graphs/core
  across 8 NeuronCores (== 2 batch items per core, since 120 graphs = one clip).
- On host: build the dense normalized adjacency Ahat (G,22,22) from
  edge_index/edge_attr (GCN norm: D^-1/2 (A+I) D^-1/2), then pack 5 graphs per
  128-partition tile as a 110x110 block-diagonal stationary (transposed).
  Layer-1 W is folded on host (ship XW1 = x @ W1).
- All inputs ride in THREE DRAM tensors / 6 DMAs (HWDGE dispatch is a fixed
  ~625ns serialization per DMA, so DMA count dominates startup); big110 is
  packed as four self-sufficient [XW1_s | AhatT_s] 12-chunk slices so each
  arriving DMA immediately enables its chunks' layer-0 AND layer-1 work:
    big110 (110, 6053) = 4x [XW1 slice | block-diag AhatT slice] + pool mat
    c128   (128, 1880) = [conv w taps | W2/W3/W4 replicated at partition
                          bases 0/32/64 for PSUM-stacked mm2]
    bn     (120, 2)    = [bn_scale/2 | bn_shift/2] f32
- A dummy-matmul warm-up train during the DMA wait keeps the PE p-state
  clock ramping so real layers run at full speed (0.42ns/col) from go.
- Per chunk c and GCN layer l (software-pipelined with 2-bank lookahead,
  flush-first PE ordering, and cross-layer overlap via held tail groups):
    mm1: M1t = H_c^T AhatT_c  -> PSUM, with 3 (cin<=32) or 2 (cin=64) chunks
         STACKED on the partition axis at PE-legal bases 0/32/64 (column
         tiling): vector eviction cost is per-column, so stacking cuts the
         M1 eviction ~3x. For cin=16 the stationary is widened to 32 so
         every evicted PSUM byte is matmul-written.
    mm2: H' = M1t^T W_l -> PSUM (110, cout), relu on eviction. Reading the
         stacked M1 at base 32q is PE ROW tiling: different row tiles run
         concurrently and must not write the same PSUM bank (hardware race
         -> device wedge), so each q gets its own PSUM bank and the H
         eviction is per-q with a strided chunk view of h_next.
  All matmuls bf16 with f32 PSUM. Evictions are spread across ACT/DVE via a
  greedy cost balancer (GPSIMD/Pool cannot access PSUM on TRN2).
- Head (interleaved into layer 3 as its chunks are evicted): mean-pool as
  matmul vs 0/1 pool matrix (1/22 folded into conv weights), conv1d(k=3) as
  shifted-lhsT matmuls, then BN(eval)+sigmoid rewritten via
  sigmoid(z)-0.5 = tanh(z/2)/2 so the whole kernel needs only ONE
  activation-table set (tanh/relu/copy) warm-loaded during the DMA window,
  plus a single tail swap for the final Sqrt (both clips' sqrts are merged
  at the very end). One packed output DMA.
"""

import os
from contextlib import ExitStack

import numpy as np

import concourse.bass as bass
import concourse.bacc as bacc
import concourse.tile as tile
from concourse import mybir
from concourse.bass_utils import run_bass_kernel_spmd

# ---- problem constants (hardcoded; kernel.py must be self-contained) ----
BS, T, P, G = 16, 120, 22, 1920
NCORES = 8
GPC = G // NCORES          # 240 graphs per core
CPG = 5                    # graphs per 128-partition chunk
CH = CPG * P               # 110 nodes per chunk
NCHUNK = GPC // CPG        # 48 chunks per core
BPC = BS // NCORES         # 2 batch items (clips) per core
KPB = T // CPG             # 24 chunks per clip
C_IN = 14
CHS = [16, 32, 64, 152]
DIMS = [C_IN] + CHS
NCLS, DIM_CAP = 17, 16
C_CONV = DIM_CAP * NCLS    # 272
BN_EPS = 1e-3

# big110 column layout: four self-sufficient 12-chunk slices, each
# [XW1 block (192) | ahat block-diag (1320)], so every DMA slice enables its
# chunks' layer-0 AND layer-1 work with no cross-slice dependency; pool
# matrix rides the last slice. One slice per DMA = compute rides the wave.
SLC = 12                             # chunks per slice
NSL = NCHUNK // SLC                  # 4 slices
SL_XW = SLC * CHS[0]                 # 192
SL_COLS = SL_XW + SLC * CH           # 1512
PM_OFS = NSL * SL_COLS               # 6048
BIG_COLS = PM_OFS + CPG              # 6053
# c128 column layout
WC1_OFS = 0                          # (128, 3*272)
WC2_OFS = 816                        # rows 0..24: (24, 3*272)
W_OFS = {1: 1632, 2: 1664, 3: 1728}  # replicated W2/W3/W4
C128_COLS = 1880

F32 = mybir.dt.float32
BF16 = mybir.dt.bfloat16
NPBF16 = np.dtype(mybir.dt.np(BF16))

TRACE = os.environ.get("KTRACE", "0") == "1"
LAST = None  # last BassKernelResults, for test harness introspection
LAST_NC = None  # last built bass.Bass module, for cost-model simulation


def _host_prep(x, edge_index, edge_attr, conv_w, W1):
    """Dense normalized adjacency + packed per-core / shared DRAM images."""
    src = np.asarray(edge_index[0], np.int64)
    dst = np.asarray(edge_index[1], np.int64)
    w = np.asarray(edge_attr[:, 4], np.float32)

    A = np.zeros((G, P, P), np.float32)
    np.add.at(A, (dst // P, dst % P, src % P), w)
    deg = A.sum(axis=2) + 1.0                      # + self-loop weight 1
    dinv = 1.0 / np.sqrt(deg)                      # deg >= 1 always
    Ahat = dinv[:, :, None] * A * dinv[:, None, :]
    ii = np.arange(P)
    Ahat[:, ii, ii] += dinv * dinv                 # self loop: dinv[d]^2
    AhatT = np.ascontiguousarray(Ahat.transpose(0, 2, 1))  # [g, s, d]

    # block-diag pack: (NCORES, CH, NCHUNK*CH); rows = source node in chunk,
    # cols = chunk*CH + dest node in chunk
    bd = np.zeros((NCORES, CH, NCHUNK * CH), np.float32)
    bdv = bd.reshape(NCORES, CH, NCHUNK, CH)
    Ar = AhatT.reshape(NCORES, NCHUNK, CPG, P, P)
    for j in range(CPG):
        bdv[:, j * P:(j + 1) * P, :, j * P:(j + 1) * P] = \
            Ar[:, :, j].transpose(0, 2, 1, 3)

    # layer-1 W folded on host: ship XW1 = x @ W1
    xw = np.asarray(x, np.float32) @ np.asarray(W1, np.float32)
    xr = xw.reshape(NCORES, NCHUNK, CH, CHS[0])
    xp = xr.transpose(0, 2, 1, 3)                  # (core, 110, 48, 16)

    big = np.zeros((NCORES, CH, BIG_COLS), np.float32)
    bdv2 = bd.reshape(NCORES, CH, NSL, SLC * CH)
    for s in range(NSL):
        o = s * SL_COLS
        big[:, :, o:o + SL_XW] = \
            xp[:, :, s * SLC:(s + 1) * SLC].reshape(NCORES, CH, SL_XW)
        big[:, :, o + SL_XW:o + SL_COLS] = bdv2[:, :, s]
    for j in range(CPG):
        big[:, j * P:(j + 1) * P, PM_OFS + j] = 1.0
    return big.astype(NPBF16)


def _host_c128(conv_w, W2, W3, W4):
    c128 = np.zeros((128, C128_COLS), np.float32)
    # conv weights (co, ci, k) -> (ci, k, co), with the 1/22 mean-pool factor
    wct = np.asarray(conv_w, np.float32).transpose(1, 2, 0) / float(P)  # (152,3,272)
    c128[:, WC1_OFS:WC1_OFS + 816] = wct[:128].reshape(128, 816)
    c128[:24, WC2_OFS:WC2_OFS + 816] = wct[128:].reshape(24, 816)
    for l, W in ((1, W2), (2, W3), (3, W4)):
        cin, cout = DIMS[l], DIMS[l + 1]
        rows = 64 if cin > 32 else 32
        for q in range(128 // rows):
            c128[rows * q:rows * q + cin, W_OFS[l]:W_OFS[l] + cout] = \
                np.asarray(W, np.float32)
    return c128.astype(NPBF16)


class _Balancer:
    """Greedy PSUM->SBUF eviction spreader. GPSIMD/Pool cannot access PSUM
    on TRN2, so evictions only go to ACT / DVE; Pool is tracked for the
    SBUF-only work it takes (memsets, squares)."""
    SPEED = {"act": 0.8333, "dve": 1.0416, "pool": 1.3889}
    FIX = {"act": 200.0, "dve": 180.0, "pool": 140.0}

    def __init__(self, nc):
        self.nc = nc
        self.t = {"act": 0.0, "dve": 0.0, "pool": 0.0}

    def _pick(self, cols, prefer=None):
        if prefer is None:
            e = min(("act", "dve"),
                    key=lambda k: self.t[k] + cols * self.SPEED[k] + self.FIX[k])
        else:
            e = prefer
        self.t[e] += cols * self.SPEED[e] + self.FIX[e]
        return e

    def charge(self, eng, cols):
        self.t[eng] += cols * self.SPEED[eng] + self.FIX[eng]

    def relu(self, dst, src, cols, prefer=None):
        e = self._pick(cols, prefer)
        if e == "act":
            self.nc.scalar.activation(dst, src, mybir.ActivationFunctionType.Relu)
        elif e == "dve":
            self.nc.vector.tensor_scalar_max(dst, src, 0.0)
        else:
            self.nc.gpsimd.tensor_scalar_max(dst, src, 0.0)

    def copy(self, dst, src, cols, prefer=None):
        e = self._pick(cols, prefer)
        if e == "act":
            self.nc.scalar.activation(dst, src, mybir.ActivationFunctionType.Copy)
        elif e == "dve":
            self.nc.vector.tensor_copy(dst, src)
        else:
            self.nc.gpsimd.tensor_copy(dst, src)


def _build(nonzero_b, nonzero_convb):
    """Build the SPMD Bass program (identical on all 8 cores)."""
    nc = bacc.Bacc()
    AF = mybir.ActivationFunctionType

    d_big = nc.declare_dram_parameter("big110", [CH, BIG_COLS], BF16, isOutput=False)
    d_c128 = nc.declare_dram_parameter("c128", [128, C128_COLS], BF16, isOutput=False)
    d_bn = nc.declare_dram_parameter("bn", [T, 2], F32, isOutput=False)
    d_b = [nc.declare_dram_parameter(f"b{l}", [1, DIMS[l + 1]], BF16, isOutput=False)
           if nonzero_b[l] else None for l in range(4)]
    d_convb = (nc.declare_dram_parameter("convb", [1, C_CONV], BF16, isOutput=False)
               if nonzero_convb else None)
    d_out = nc.declare_dram_parameter("out", [T, BPC * NCLS], F32, isOutput=True)

    with tile.TileContext(nc) as tc, ExitStack() as ctx:
        const = ctx.enter_context(tc.tile_pool(name="const", bufs=1))
        state = ctx.enter_context(tc.tile_pool(name="state", bufs=1))
        m1p = ctx.enter_context(tc.tile_pool(name="m1sb", bufs=4))
        psM = ctx.enter_context(tc.tile_pool(name="psM", bufs=3, space="PSUM"))
        psH = ctx.enter_context(tc.tile_pool(name="psH", bufs=4, space="PSUM"))
        psPT = ctx.enter_context(tc.tile_pool(name="psPT", bufs=1, space="PSUM"))
        hd = ctx.enter_context(tc.tile_pool(name="head", bufs=2))

        bal = _Balancer(nc)

        # ---- warm the activation table set during the DMA wait window ----
        # First ACT instruction is a Tanh so the table-load pass picks a set
        # containing tanh (every tanh set also has relu/copy/square): all ACT
        # work except the final Sqrt runs under one set with zero reloads.
        t_ones = const.tile([1, 128], BF16)
        nc.gpsimd.memset(t_ones, 1.0)
        if os.environ.get("KACTWARM", "1") == "1":
            t_warm = const.tile([1, 1], F32)
            nc.gpsimd.memset(t_warm, 0.0)
            nc.scalar.activation(t_warm, t_warm, AF.Tanh)

        # ---- input DMAs (6 total; HWDGE dispatch is ~625ns each), ordered by
        # first use so compute starts as early as possible ----
        # Slice 0 is split (xw + first 6 ahat chunks) so the first matmul
        # starts ~0.5us earlier; c128 (W replicas) must land before the first
        # mm2 flush (~6us), so its transfer is sequenced after slice 1.
        bounds = [0] + [(s + 1) * SL_COLS for s in range(NSL)]
        bounds[-1] += CPG
        t_parts = []
        t_c128 = None
        for i in range(len(bounds) - 1):
            o, e = bounds[i], bounds[i + 1]
            tp = const.tile([CH, e - o], BF16, name=f"slc{i}", tag=f"slc{i}")
            nc.sync.dma_start(out=tp, in_=d_big[:, o:e])
            t_parts.append((o, e, tp))
            if i == 2:
                t_c128 = const.tile([128, C128_COLS], BF16)  # conv w + W reps
                nc.sync.dma_start(out=t_c128, in_=d_c128[:])
        t_bn = const.tile([T, 2], F32)
        nc.sync.dma_start(out=t_bn, in_=d_bn[:])
        t_b = []
        for l in range(4):
            if d_b[l] is not None:
                tb = const.tile([1, DIMS[l + 1]], BF16, tag=f"b{l}")
                nc.sync.dma_start(out=tb, in_=d_b[l][:])
                t_b.append(tb)
            else:
                t_b.append(None)
        if d_convb is not None:
            t_convb = const.tile([1, C_CONV], BF16)
            nc.sync.dma_start(out=t_convb, in_=d_convb[:])

        def big_cols(c0, c1):
            for o, e, tp in t_parts:
                if o <= c0 and c1 <= e:
                    return tp[:, c0 - o:c1 - o]
            raise AssertionError((c0, c1))

        def ahat_chunk(k):
            s, j = divmod(k, SLC)
            o = s * SL_COLS + SL_XW + j * CH
            return big_cols(o, o + CH)

        def xw_chunk(k):
            s, j = divmod(k, SLC)
            o = s * SL_COLS + j * 16
            return big_cols(o, o + 16)

        t_poolm = big_cols(PM_OFS, PM_OFS + CPG)  # (110, 5)

        # h0 gets one extra (zeroed) chunk so layer-1's widened stationary
        # slices stay in bounds for the last chunk
        h_tiles = [state.tile([CH, NCHUNK + (1 if l == 0 else 0), DIMS[l + 1]],
                              BF16, tag=f"h{l}", name=f"h{l}")
                   for l in range(4)]

        # ---- PE p-state warm-up: a dummy matmul train during the DMA wait
        # keeps the tensor engine's ramp clock running so the real layers
        # start at full speed (0.42ns/col) instead of the mid p-state; sized
        # to end just as the first DMA slice lands ----
        if os.environ.get("KWARM", "1") == "1":
            ps_warm = psPT.tile([128, 512], F32, tag="pt", name="warmps")
            for _ in range(int(os.environ.get("KWARMN", "28"))):
                nc.tensor.matmul(ps_warm[:, :128], lhsT=t_ones, rhs=t_ones,
                                 start=True, stop=True)

        # pad chunk for layer-1's widened mm1 stationary (see gcn_layer)
        nc.gpsimd.memset(h_tiles[0][:, NCHUNK, :], 0.0)

        # ---- layer 0: H1 = relu(Ahat @ XW1 (+ b1)); bank boundaries aligned
        # with the DMA slices (first slice is half-size) so compute rides
        # the wave ----
        L0BOUNDS = [0, 12, 24, 36, 48]
        def l0_bank(g):
            c0, c1 = L0BOUNDS[g], L0BOUNDS[g + 1]
            ps = psH.tile([128, 512], F32, tag="h")
            for j, k in enumerate(range(c0, c1)):
                nc.tensor.matmul(
                    ps[:CH, j * 16:(j + 1) * 16],
                    lhsT=ahat_chunk(k), rhs=xw_chunk(k),
                    start=True, stop=(t_b[0] is None))
                if t_b[0] is not None:
                    nc.tensor.matmul(
                        ps[:CH, j * 16:(j + 1) * 16],
                        lhsT=t_ones[:, :CH], rhs=t_b[0][:],
                        start=False, stop=True, skip_group_check=True)
            dst = h_tiles[0][:, c0:c1, :].rearrange("p a b -> p (a b)")
            # layer-0 evictions gate layer-1 mm1 while riding the DMA wave
            # (all engines idle then): pin to the fastest engine
            bal.relu(dst, ps[:CH, :(c1 - c0) * 16], (c1 - c0) * 16, prefer="act")

        # ---- layers 1-3: PSUM-stacked mm1, W-replica mm2, software-pipelined ----
        def gcn_layer(l, inject=None, on_progress=None, after_first_bank=None,
                      hold_chunks=0):
            cin, cout = DIMS[l], DIMS[l + 1]
            rows = 64 if cin > 32 else 32
            stacks = 2 if cin > 32 else 3     # AP partition bases: 0/32/64 only
            if os.environ.get("KSTACK", "1") == "0":
                stacks = 1
            cpb = stacks * 4                  # chunks per mm1 PSUM bank
            gsz = 512 // cout                 # chunks per mm2 PSUM bank
            nbanks = (NCHUNK + cpb - 1) // cpb
            ngroups = (NCHUNK + gsz - 1) // gsz
            h_prev, h_next = h_tiles[l - 1], h_tiles[l]
            m1sb = {}
            pend = [0]

            # mm2 reads the stacked M1 at partition base 32q => PE ROW tiling.
            # Different row tiles run concurrently in the array and MUST NOT
            # write the same PSUM bank (HW race -> device wedge), so each q
            # gets its own PSUM bank; the H eviction is per-q with a strided
            # chunk view of h_next.
            m_per_q = 512 // cout // (2 if stacks == 3 and cout <= 32 else 1)
            gsz = min(m_per_q, (NCHUNK // stacks + 2)) * stacks
            gsz = min(gsz, 512 // cout * stacks)
            ngroups = (NCHUNK + gsz - 1) // gsz
            hv = h_next[:, :, :].rearrange("p (t s) c -> p t s c", s=stacks) \
                if stacks > 1 else None

            def flush(upto):
                while pend[0] < ngroups and min((pend[0] + 1) * gsz, NCHUNK) <= upto:
                    g = pend[0]
                    c0, c1 = g * gsz, min((g + 1) * gsz, NCHUNK)
                    qb = [psH.tile([128, 512], F32, tag="h", name=f"hq{q}")
                          for q in range(stacks)]
                    for k in range(c0, c1):
                        b, jj = divmod(k, cpb)
                        q, t = jj % stacks, (k - c0) // stacks
                        nc.tensor.matmul(
                            qb[q][:CH, t * cout:(t + 1) * cout],
                            lhsT=m1sb[b][rows * q:rows * q + cin,
                                         (jj // stacks) * CH:(jj // stacks + 1) * CH],
                            rhs=t_c128[rows * q:rows * q + cin,
                                       W_OFS[l]:W_OFS[l] + cout],
                            start=True, stop=(t_b[l] is None))
                        if t_b[l] is not None:
                            nc.tensor.matmul(
                                qb[q][:CH, t * cout:(t + 1) * cout],
                                lhsT=t_ones[:, :CH], rhs=t_b[l][:],
                                start=False, stop=True, skip_group_check=True)
                    m = (c1 - c0) // stacks
                    for q in range(stacks):
                        if stacks > 1:
                            dst = hv[:, c0 // stacks:c0 // stacks + m, q, :]
                            src = qb[q][:CH, :m * cout].rearrange(
                                "p (a b) -> p a b", b=cout)
                        else:
                            dst = h_next[:, c0:c1, :].rearrange("p a b -> p (a b)")
                            src = qb[q][:CH, :m * cout]
                        # layer-3 H evictions gate the head pool matmuls:
                        # keep them on the fast engines, alternating
                        pref = ("act" if (g * stacks + q) % 2 == 1 else "dve") \
                            if on_progress is not None else None
                        bal.relu(dst, src, m * cout, prefer=pref)
                    pend[0] += 1
                    if on_progress is not None:
                        on_progress(min(pend[0] * gsz, NCHUNK))

            # Every evicted PSUM byte must be matmul-written: with 3 stacks
            # only rows [0,96) are used, and for cin=16 the stationary is
            # widened to 32 columns (next chunk's h rides along as a write-
            # only filler the mm2 never reads) so the 32-row bands are full.
            wid = rows if (l == 1 and stacks > 1) else cin
            ev_rows = rows * (stacks - 1) + wid
            for b in range(nbanks):
                # flush-first: eligible mm2 groups go ahead of a possibly
                # DMA-stalled mm1 bank in the in-order PE queue
                flush(max(0, (b - 1) * cpb))   # 2-bank software-pipeline lookahead
                if inject is not None:
                    inject(min((b + 1) * cpb + (1 if l == 1 else 0), NCHUNK))
                c0, c1 = b * cpb, min((b + 1) * cpb, NCHUNK)
                psm = psM.tile([128, 512], F32, tag="m")
                for jj, k in enumerate(range(c0, c1)):
                    q, cg = jj % stacks, jj // stacks
                    lhsT = (h_prev[:, k, :] if wid == cin else
                            h_prev[:, k:k + 2, :].rearrange("p a b -> p (a b)"))
                    nc.tensor.matmul(
                        psm[rows * q:rows * q + wid, cg * CH:(cg + 1) * CH],
                        lhsT=lhsT, rhs=ahat_chunk(k),
                        start=True, stop=True)
                sb = m1p.tile([128, 4 * CH], BF16, tag="msb")
                bal.copy(sb[:ev_rows], psm[:ev_rows, :4 * CH], 4 * CH)
                m1sb[b] = sb
                if b == 0 and after_first_bank is not None:
                    after_first_bank()
            flush(NCHUNK - hold_chunks)

            def finish():
                flush(NCHUNK)
            return finish

        l0_done = [0]

        def l0_upto(chunk_end):
            while l0_done[0] < len(L0BOUNDS) - 1 and \
                    L0BOUNDS[l0_done[0]] < min(chunk_end, NCHUNK):
                l0_bank(l0_done[0])
                l0_done[0] += 1

        # ---- per-clip head, interleaved into layer 3 via on_progress ----
        t_q = const.tile([T, BPC * NCLS], F32)
        t_y = const.tile([T, BPC * NCLS], F32)
        pool_done = [0]
        pt_banks = {}

        pt_sb = {}

        def head_pt(b):
            ps1, ps2 = pt_banks.pop(b)
            # zero-padded columns at both ends so conv shifts stay PE-legal
            pt1 = hd.tile([128, T + 2], BF16, tag="pt1")
            pt2 = hd.tile([CHS[3] - 128, T + 2], BF16, tag="pt2")
            # pt evictions gate the conv matmuls: run the two in parallel on
            # the fast engines
            for pt, ps, np_, pref in ((pt1, ps1, 128, "act"),
                                      (pt2, ps2, CHS[3] - 128, "dve")):
                nc.gpsimd.memset(pt[:, 0:1], 0.0)
                nc.gpsimd.memset(pt[:, T + 1:T + 2], 0.0)
                bal.copy(pt[:, 1:T + 1], ps[:np_, :T], T, prefer=pref)
            pt_sb[b] = (pt1, pt2)

        def head_tail(b):
            pt1, pt2 = pt_sb.pop(b)

            # conv over T: out[t] = sum_k w[k] @ feat[t+k-1], zero-padded
            ps_caps = psH.tile([128, 512], F32, tag="h")
            first = True
            for ci, (pt, wofs, kcin) in enumerate(
                    ((pt1, WC1_OFS, 128), (pt2, WC2_OFS, CHS[3] - 128))):
                for kk in range(3):
                    last = (d_convb is None) and ci == 1 and kk == 2
                    nc.tensor.matmul(
                        ps_caps[:T, :C_CONV], lhsT=pt[:, kk:kk + T],
                        rhs=t_c128[:kcin, wofs + kk * C_CONV:wofs + (kk + 1) * C_CONV],
                        start=first, stop=last, skip_group_check=True)
                    first = False
            if d_convb is not None:
                nc.tensor.matmul(ps_caps[:T, :C_CONV], lhsT=t_ones[:, :T],
                                 rhs=t_convb[:], start=False, stop=True,
                                 skip_group_check=True)

            # sigmoid(z)-0.5 == tanh(z/2)/2: u = tanh(caps*bnscale/2 + bnshift/2)
            u = hd.tile([T, C_CONV], BF16, tag="u")
            nc.scalar.activation(u, ps_caps[:T, :C_CONV], AF.Tanh,
                                 bias=t_bn[:, 1:2], scale=t_bn[:, 0:1])
            bal.charge("act", C_CONV)
            # q = sum_d u^2 ; out = sqrt(q/16) == sqrt(sum (s-.5)^2 * 4/16)
            sq = hd.tile([T, C_CONV], BF16, tag="sq")
            use_pool_sq = b == 0 and os.environ.get("KPOOLSQ", "1") == "1"
            veng = nc.gpsimd if use_pool_sq else nc.vector
            veng.tensor_mul(sq, u, u)
            bal.charge("pool" if use_pool_sq else "dve", C_CONV)
            nc.vector.reduce_sum(
                out=t_q[:, b * NCLS:(b + 1) * NCLS],
                in_=sq.rearrange("p (d c) -> p c d", c=NCLS),
                axis=mybir.AxisListType.X)
            bal.charge("dve", C_CONV)

        ht_pending = [None]
        pt_pending = [None]

        def head_progress(evicted):
            # Lag pool matmuls one eviction group behind so they never
            # head-block the PE queue on a just-emitted H eviction; same for
            # each clip's conv/tanh tail (deferred one more group).
            if pt_pending[0] is not None and evicted >= pt_pending[0][1]:
                b, pt_pending[0] = pt_pending[0][0], None
                head_pt(b)
            if ht_pending[0] is not None and evicted >= ht_pending[0][1]:
                b, ht_pending[0] = ht_pending[0][0], None
                head_tail(b)
            lagged = evicted - 6 if evicted < NCHUNK else NCHUNK
            # pooledT built directly: lhsT = H4 chunk, rhs = pool 0/1 matrix
            while pool_done[0] < min(lagged, NCHUNK):
                k = pool_done[0]
                b, kk = divmod(k, KPB)
                if kk == 0:
                    # pt1 (128,120) and pt2 (24,120) share one PSUM bank at
                    # disjoint column ranges
                    ptb = psPT.tile([128, 512], F32, tag="pt", name="ptA")
                    pt_banks[b] = (ptb[:, :128], ptb[:, 128:256])
                ps1, ps2 = pt_banks[b]
                nc.tensor.matmul(ps1[:, kk * CPG:(kk + 1) * CPG],
                                 lhsT=h_tiles[3][:, k, :128], rhs=t_poolm,
                                 start=True, stop=True)
                nc.tensor.matmul(ps2[:CHS[3] - 128, kk * CPG:(kk + 1) * CPG],
                                 lhsT=h_tiles[3][:, k, 128:], rhs=t_poolm,
                                 start=True, stop=True)
                pool_done[0] += 1
                if pool_done[0] % KPB == 0:
                    # stage the head: pt copies soon (they only occupy the
                    # eviction engines), conv/tanh later (so they never
                    # head-block PE)
                    pt_pending[0] = (pool_done[0] // KPB - 1,
                                     min(pool_done[0] + 6, NCHUNK))
                    ht_pending[0] = (pool_done[0] // KPB - 1,
                                     min(pool_done[0] + 24, NCHUNK))

        l0_upto(12)
        fin1 = gcn_layer(1, inject=l0_upto, hold_chunks=24)
        fin2 = gcn_layer(2, after_first_bank=fin1, hold_chunks=24)
        gcn_layer(3, on_progress=head_progress, after_first_bank=fin2)
        head_progress(NCHUNK)
        if pt_pending[0] is not None:
            head_pt(pt_pending[0][0])
        if ht_pending[0] is not None:
            head_tail(ht_pending[0][0])

        # one joint sqrt for both clips: exactly one act-table swap at the tail
        nc.scalar.activation(t_y, t_q, AF.Sqrt, scale=1.0 / DIM_CAP)
        nc.sync.dma_start(out=d_out[:], in_=t_y)

    return nc


def kernel(x, edge_index, batch, edge_attr, W1, b1, W2, b2, W3, b3, W4, b4,
           conv_w, conv_b, bn_gamma, bn_beta):
    global LAST, LAST_NC
    big = _host_prep(x, edge_index, edge_attr, conv_w, W1)
    c128 = _host_c128(conv_w, W2, W3, W4)

    bs = [np.asarray(b_, np.float32) for b_ in (b1, b2, b3, b4)]
    nonzero_b = [bool(np.any(b_)) for b_ in bs]
    convb = np.asarray(conv_b, np.float32)
    nonzero_convb = bool(np.any(convb))

    bn = np.stack([
        np.asarray(bn_gamma, np.float32) / np.sqrt(1.0 + BN_EPS) * 0.5,
        np.asarray(bn_beta, np.float32) * 0.5,
    ], axis=1)  # (T, 2) f32: [bnscale/2, bnshift/2]

    nc = _build(nonzero_b, nonzero_convb)
    if not nc.is_finalized():
        nc.finalize()   # Bacc: runs the wait-splitting/regalloc compile passes
    LAST_NC = nc

    in_maps = []
    for c in range(NCORES):
        m = dict(big110=np.ascontiguousarray(big[c]), c128=c128, bn=bn)
        for l in range(4):
            if nonzero_b[l]:
                m[f"b{l}"] = bs[l].reshape(1, -1).astype(NPBF16)
        if nonzero_convb:
            m["convb"] = convb.reshape(1, -1).astype(NPBF16)
        in_maps.append(m)

    LAST = run_bass_kernel_spmd(nc, in_maps, core_ids=list(range(NCORES)),
                                trace=TRACE)
    out = np.empty((BS, T, NCLS), np.float32)
    for c in range(NCORES):
        yc = LAST.results[c]["out"]          # (T, BPC*NCLS)
        for b in range(BPC):
            out[c * BPC + b] = yc[:, b * NCLS:(b + 1) * NCLS]
    return out
